# revision 2
# baseline (speedup 1.0000x reference)
"""Trainium2 Bass kernel for DiagonalMultiplySum.

out[b, o, s] = sum_i input[b, i, s] * diagonal[o, i, s]

Shapes (hardcoded): input (64, 256, 4096) f32, diagonal (256, 256, 4096) f32,
output (64, 256, 4096) f32.

Strategy (v2, bf16):
- Shard the size axis across 8 NeuronCores (512 positions per core); every
  position s is an independent matmul  out[:, :, s] = in[:, :, s] @ diag[:, :, s].T
  contracted over i (256 -> 2 chunks of 128 on the PE partition dim).
- The kernel is HBM-bound: compulsory traffic is diag (dominant) + input + out.
  All tensors are cast to bf16 on the HOST and uploaded pre-packed in the exact
  SBUF layout, halving HBM traffic vs fp32 (fp32 floor ~562us/core, bf16
  ~281us/core at 358 GB/s) while keeping rel-err ~3e-3 (gate 2e-2, fp32 PSUM
  accumulation).
- Per position: input is the stationary operand [K=128 i, M=64 b] (cheap
  64-column LDWEIGHTS), diagonal is the moving operand [K=128 i, N=256 o];
  the two i-chunks accumulate in PSUM [b=64, o=256].  Two positions share one
  PSUM bank; DVE drains each bank (fp32->bf16) into an SBUF staging tile.
- s-windows of W=32 positions, double buffered.  Loads (input+diag) ride the
  SP HWDGE ring (nc.sync), stores ride the ACT ring (nc.scalar) so a store
  never head-of-line-blocks a load.
"""

import os
import sys

for _p in ("/opt/trn_rl_repo",):
    if _p not in sys.path and os.path.isdir(_p):
        sys.path.insert(0, _p)

import numpy as np

BATCH = 64
OUT_C = 256
IN_C = 256
SIZE = 4096
N_CORES = 8
S = SIZE // N_CORES  # 512 positions per core
P = 128

W = int(os.environ.get("DMS_W", "32"))  # positions per window
NW = S // W

FREE_IN = W * 2 * BATCH  # per-partition elems per window: [s][ic][b]
FREE_DG = W * 2 * OUT_C  # per-partition elems per window: [s][ic][o]
FREE_OUT = W * OUT_C  # per-partition elems per window: [s][o]

_NC_CACHE = {}


def _build_nc():
    import concourse.bass as bass
    import concourse.mybir as mybir
    import concourse.tile as tile
    from contextlib import ExitStack

    fp32 = mybir.dt.float32
    bf16 = mybir.dt.bfloat16
    nc = bass.Bass(trn_type="TRN2")

    # Pre-packed DRAM layouts (packed on host, see kernel()):
    #   input:    [p, (w s ic b)]  p = i % 128, ic = i // 128
    #   diagonal: [p, (w s ic o)]
    #   output:   [b, (w s o)]     (64 partitions)
    inp = nc.dram_tensor("input", [P, NW * FREE_IN], bf16, kind="ExternalInput")
    dg = nc.dram_tensor("diagonal", [P, NW * FREE_DG], bf16, kind="ExternalInput")
    out = nc.dram_tensor("output", [BATCH, NW * FREE_OUT], bf16, kind="ExternalOutput")

    with tile.TileContext(nc) as tc, ExitStack() as ctx:
        in_pool = ctx.enter_context(tc.tile_pool(name="inp", bufs=2))
        dg_pool = ctx.enter_context(tc.tile_pool(name="dgp", bufs=2))
        out_pool = ctx.enter_context(tc.tile_pool(name="outp", bufs=2))
        ps_pool = ctx.enter_context(tc.tile_pool(name="psp", bufs=8, space="PSUM"))

        tiles = {}

        def load(w):
            in_t = in_pool.tile([P, FREE_IN], bf16, name="in_t", tag="in_t")
            nc.sync.dma_start(out=in_t, in_=inp[:, w * FREE_IN : (w + 1) * FREE_IN])
            dg_t = dg_pool.tile([P, FREE_DG], bf16, name="dg_t", tag="dg_t")
            nc.sync.dma_start(out=dg_t, in_=dg[:, w * FREE_DG : (w + 1) * FREE_DG])
            tiles[w] = (in_t, dg_t)

        load(0)
        for w in range(NW):
            if w + 1 < NW:
                load(w + 1)
            in_t, dg_t = tiles.pop(w)

            # compute views
            in_t4 = in_t.rearrange("p (s ic b) -> p s ic b", ic=2, b=BATCH)
            dg_t4 = dg_t.rearrange("p (s ic o) -> p s ic o", ic=2, o=OUT_C)

            out_t = out_pool.tile([BATCH, FREE_OUT], bf16, name="out_t")
            out_t3 = out_t.rearrange("p (s o) -> p s o", o=OUT_C)

            for pair in range(W // 2):
                ps = ps_pool.tile([BATCH, 2 * OUT_C], fp32, name="ps")
                for k in range(2):
                    s_loc = pair * 2 + k
                    for ic in range(2):
                        nc.tensor.matmul(
                            ps[:, k * OUT_C : (k + 1) * OUT_C],
                            in_t4[:, s_loc, ic, :],
                            dg_t4[:, s_loc, ic, :],
                            start=(ic == 0),
                            stop=(ic == 1),
                        )
                nc.vector.tensor_copy(
                    out_t3[:, pair * 2 : pair * 2 + 2, :],
                    ps.rearrange("p (s o) -> p s o", o=OUT_C),
                )

            nc.scalar.dma_start(
                out=out[:, w * FREE_OUT : (w + 1) * FREE_OUT], in_=out_t
            )

    _split_multi_waits(nc)
    return nc


def _split_multi_waits(nc):
    """Walrus codegen supports only ONE sync-wait per instruction.

    Tile emits multiple waits on some instructions; hoist all but the last
    onto same-engine NoOp instructions inserted immediately before the
    offender.  Per-engine in-order issue makes this exactly equivalent.
    """
    import concourse.mybir as mybir

    for f in nc.m.functions:
        for blk in f.blocks:
            new_list = []
            changed = False
            for inst in blk.instructions:
                si = inst.sync_info
                waits = list(si.on_wait) if si and si.on_wait else []
                if len(waits) > 1:
                    for wt in waits[:-1]:
                        nop = mybir.InstNoOp(
                            name=nc.get_next_instruction_name(),
                            engine=inst.engine,
                            ins=[],
                            outs=[],
                            sync_info=mybir.SyncInfo(on_wait=[wt], on_update=[]),
                        )
                        nc.register_instruction(nop)
                        new_list.append(nop)
                    si.on_wait = [waits[-1]]
                    changed = True
                new_list.append(inst)
            if changed:
                blk.instructions = new_list


def _get_nc():
    key = "nc"
    if key not in _NC_CACHE:
        _NC_CACHE[key] = _build_nc()
    return _NC_CACHE[key]


def pack_inputs(inp, dg):
    """fp32 full tensors -> per-core pre-packed bf16 in_maps."""
    import ml_dtypes

    bf = ml_dtypes.bfloat16
    inp16 = np.asarray(inp, dtype=np.float32).astype(bf)
    dg16 = np.asarray(dg, dtype=np.float32).astype(bf)

    # input (b, i, s) -> [core, p, w, s_loc, ic, b]
    iv = inp16.reshape(BATCH, 2, P, N_CORES, NW, W)
    ipk = np.ascontiguousarray(iv.transpose(3, 2, 4, 5, 1, 0)).reshape(
        N_CORES, P, NW * FREE_IN
    )
    # diagonal (o, i, s) -> [core, p, w, s_loc, ic, o]
    dv = dg16.reshape(OUT_C, 2, P, N_CORES, NW, W)
    dpk = np.ascontiguousarray(dv.transpose(3, 2, 4, 5, 1, 0)).reshape(
        N_CORES, P, NW * FREE_DG
    )
    return [
        {"input": ipk[c], "diagonal": dpk[c]} for c in range(N_CORES)
    ]


def unpack_output(results):
    """Per-core packed bf16 outputs -> full fp32 (64, 256, 4096)."""
    outs = []
    for c in range(N_CORES):
        o = np.asarray(results[c]["output"]).reshape(BATCH, NW, W, OUT_C)
        outs.append(o.transpose(0, 3, 1, 2).reshape(BATCH, OUT_C, S))
    return np.concatenate(outs, axis=2).astype(np.float32)


def kernel(**inputs):
    inp = inputs["input"]
    dg = inputs["diagonal"]
    assert tuple(inp.shape) == (BATCH, IN_C, SIZE), inp.shape
    assert tuple(dg.shape) == (OUT_C, IN_C, SIZE), dg.shape

    from concourse.bass_utils import run_bass_kernel_spmd

    nc = _get_nc()
    in_maps = pack_inputs(inp, dg)
    res = run_bass_kernel_spmd(nc, in_maps, list(range(N_CORES)))
    return unpack_output(res.results)
